# revision 1
# baseline (speedup 1.0000x reference)
"""GatedDeltaNet (windowed gated delta rule) Trainium2 kernel.

Sharding: sequence-parallel over 8 cores. Core c handles batch c//4,
positions [1024*(c%4), 1024*(c%4)+1024). The attention window is only 4
positions back, so each core needs a 4-row halo of x on the left; no
collectives are needed.

Per-core pipeline (all in one SPMD Bass program):
  - xT (pre-transposed x slice, [2048, 1028]) resident in SBUF as 16 k-tiles
  - QK projection in "layout b" ([feat, pos]) with host-permuted weight
    columns so RoPE pairs (even/odd dims) land in separate 128-partition
    tiles, two heads per tile. RoPE applied with 6 DVE ops per tile pair.
  - Banded scores: per 128-position tile, S[j,i] for the 5-diagonal band via
    two K=64 PE matmuls (even+odd halves) against a 128-wide shifted j
    window, plus a [4, 128] corner block. Gate sigmoid(a_i*b_j) computed on
    ACT with per-partition scale (b_j columns obtained via PE transpose of
    the gate projections), then A = S * G * bandmask on DVE.
  - Attention output directly in layout b via out^T[d,i] = sum_j V[j,d]A[j,i]
    (V tiles in layout a act as the stationary operand).
  - Gated RMSNorm fused: y = out*sigmoid(z); sum of squares via ones-vector
    matmul; 1/rms applied at the final PSUM evacuation of the out-projection.
"""

import os
import sys
import functools
from contextlib import ExitStack

sys.path.insert(0, "/opt/trn_rl_repo")

import numpy as np
import ml_dtypes

import concourse.bass as bass
import concourse.mybir as mybir
import concourse.tile as tile
from concourse import bacc
from concourse.bass import ds
from concourse.bass_utils import run_bass_kernel_spmd
from concourse.masks import make_identity

F32 = mybir.dt.float32
AF = mybir.ActivationFunctionType

# Problem constants
B, S, H = 2, 4096, 2048
NH, D, W = 8, 128, 4
KD = NH * D  # 1024
ROPE_BASE = 1000000.0
EPS = 1e-6
SCALE = float(1.0 / np.sqrt(D))

NCORES = 8
SC = 1024        # owned positions per core
PH = SC + 4      # with 4-row halo: 1028
NT = SC // 128   # 8 position tiles
CHUNKS3 = [(0, 512), (512, 512), (1024, 4)]  # pos' chunks for projections

# Matmul-fabric dtype (bf16 keeps SBUF and PE happy; PSUM accumulates fp32)
DT_NAME = os.environ.get("GDN_DT", "bf16")
DT = mybir.dt.bfloat16 if DT_NAME == "bf16" else F32
NPDT = ml_dtypes.bfloat16 if DT_NAME == "bf16" else np.float32


def build_program():
    nc = bacc.Bacc("TRN2", target_bir_lowering=False, debug=False)

    # ---- DRAM I/O ----
    xT = nc.dram_tensor("xT", [16, 128, PH], DT, kind="ExternalInput")
    w_qk = nc.dram_tensor("w_qk", [16, 128, 16, 128], DT, kind="ExternalInput")
    w_v = nc.dram_tensor("w_v", [2, 128, 16, 512], DT, kind="ExternalInput")
    w_z = nc.dram_tensor("w_z", [8, 128, 16, 128], DT, kind="ExternalInput")
    w_ba = nc.dram_tensor("w_ba", [128, 16, 16], DT, kind="ExternalInput")
    w_out = nc.dram_tensor("w_out", [16, 128, 8, 128], DT, kind="ExternalInput")
    cos2_d = nc.dram_tensor("cos2", [128, PH], F32, kind="ExternalInput")
    sin2_d = nc.dram_tensor("sin2", [128, PH], F32, kind="ExternalInput")
    mask_c0_d = nc.dram_tensor("mask_c0", [128, 512], F32, kind="ExternalInput")
    mask_c1_d = nc.dram_tensor("mask_c1", [128, 512], F32, kind="ExternalInput")
    mask_add_d = nc.dram_tensor("mask_add", [128, 128], F32, kind="ExternalInput")
    mask2_d = nc.dram_tensor("mask2", [4, 512], F32, kind="ExternalInput")
    outT = nc.dram_tensor("outT", [16, 128, SC], F32, kind="ExternalOutput")

    with tile.TileContext(nc) as tc, ExitStack() as _ctx:
        const = _ctx.enter_context(tc.tile_pool(name="const", bufs=1))
        xt_pool = _ctx.enter_context(tc.tile_pool(name="xt", bufs=16))
        wqk_pool = _ctx.enter_context(tc.tile_pool(name="wqk", bufs=4))
        wv_pool = _ctx.enter_context(tc.tile_pool(name="wv", bufs=1))
        wz_pool = _ctx.enter_context(tc.tile_pool(name="wz", bufs=2))
        wo_pool = _ctx.enter_context(tc.tile_pool(name="wo", bufs=2))
        qk_pool = _ctx.enter_context(tc.tile_pool(name="qk", bufs=6))
        rt_pool = _ctx.enter_context(tc.tile_pool(name="rt", bufs=4))
        v_pool = _ctx.enter_context(tc.tile_pool(name="v", bufs=10))
        ba_pool = _ctx.enter_context(tc.tile_pool(name="ba", bufs=1))
        baat_pool = _ctx.enter_context(tc.tile_pool(name="baat", bufs=9))
        agb_pool = _ctx.enter_context(tc.tile_pool(name="agb", bufs=2))
        z_pool = _ctx.enter_context(tc.tile_pool(name="zp", bufs=2))
        y_pool = _ctx.enter_context(tc.tile_pool(name="yp", bufs=4))
        ag_pool = _ctx.enter_context(tc.tile_pool(name="ag", bufs=2))
        yn_pool = _ctx.enter_context(tc.tile_pool(name="yn", bufs=8))
        o_pool = _ctx.enter_context(tc.tile_pool(name="op", bufs=2))
        misc = _ctx.enter_context(tc.tile_pool(name="misc", bufs=1))

        dram_p = _ctx.enter_context(tc.tile_pool(name="dram", bufs=1, space="DRAM"))
        psA = _ctx.enter_context(tc.tile_pool(name="psA", bufs=4, space="PSUM"))
        psB = _ctx.enter_context(tc.tile_pool(name="psB", bufs=1, space="PSUM"))
        psC = _ctx.enter_context(tc.tile_pool(name="psC", bufs=2, space="PSUM"))

        # ---- constants ----
        cos2 = const.tile([128, PH], F32, tag="cos2")
        sin2 = const.tile([128, PH], F32, tag="sin2")
        mask_c0 = const.tile([128, 512], F32, tag="mc0")
        mask_c1 = const.tile([128, 512], F32, tag="mc1")
        mask_add = const.tile([128, 128], F32, tag="madd")
        mask2 = const.tile([4, 512], F32, tag="m2")
        ident = const.tile([16, 16], F32, tag="ident")
        ones = const.tile([128, 1], F32, tag="ones")
        epsc = const.tile([1, 1], F32, tag="epsc")
        nc.vector.memset(epsc, EPS)
        nc.sync.dma_start(out=cos2, in_=cos2_d[:, :])
        nc.sync.dma_start(out=sin2, in_=sin2_d[:, :])
        nc.sync.dma_start(out=mask_c0, in_=mask_c0_d[:, :])
        nc.sync.dma_start(out=mask_c1, in_=mask_c1_d[:, :])
        nc.sync.dma_start(out=mask_add, in_=mask_add_d[:, :])
        nc.sync.dma_start(out=mask2, in_=mask2_d[:, :])
        make_identity(nc, ident)
        nc.vector.memset(ones, 1.0)

        # ---- xT resident ----
        xt = []
        for k in range(16):
            t = xt_pool.tile([128, PH], DT, tag="xt")
            nc.sync.dma_start(out=t, in_=xT[k, :, :])
            xt.append(t)

        # ---- gate projections: BA = [bg(8); ag(8)] rows over pos' ----
        wba_sb = misc.tile([128, 16, 16], DT, tag="wba")
        nc.sync.dma_start(out=wba_sb, in_=w_ba[:, :, :])
        ba_sb = ba_pool.tile([16, PH], F32, tag="ba")
        for (st, wd) in CHUNKS3:
            bps = psA.tile([16, 512], F32, tag="ps")
            for k in range(16):
                nc.tensor.matmul(bps[:, :wd], lhsT=wba_sb[:, k, :],
                                 rhs=xt[k][:, ds(st, wd)],
                                 start=(k == 0), stop=(k == 15))
            nc.scalar.copy(ba_sb[:, ds(st, wd)], bps[:, :wd])

        ba_dram = dram_p.tile([16, PH], F32, tag="badr")
        nc.sync.dma_start(out=ba_dram, in_=ba_sb)

        # transpose gate rows -> per-tile gate columns [w, 16]
        baat = []
        for t in range(NT + 1):
            wd = 128 if t < NT else 4
            tps = psA.tile([128, 16], F32, tag="ps")
            nc.tensor.transpose(tps[:wd, :], ba_sb[:, ds(t * 128, wd)], ident)
            bt = baat_pool.tile([128, 16], F32, tag="baat")
            nc.scalar.copy(bt[:wd, :], tps[:wd, :])
            baat.append(bt)

        yn_tiles = []
        sum_ps = [psC.tile([1, 512], F32, tag="sum", name=f"sum_ps{i}")
                  for i in range(2)]

        for half in range(2):
            # ---- V projection for heads 4*half..4*half+3 (layout a) ----
            wv_sb = wv_pool.tile([128, 16, 512], DT, tag="wv")
            nc.sync.dma_start(out=wv_sb, in_=w_v[half, :, :, :])
            vtiles = []
            for t in range(NT + 1):
                wd = 128 if t < NT else 4
                vps = psA.tile([128, 512], F32, tag="ps")
                for k in range(16):
                    nc.tensor.matmul(vps[:wd, :], lhsT=xt[k][:, ds(t * 128, wd)],
                                     rhs=wv_sb[:, k, :],
                                     start=(k == 0), stop=(k == 15))
                vt = v_pool.tile([128, 512], DT, tag="v")
                nc.scalar.copy(vt[:wd, :], vps[:wd, :])
                vtiles.append(vt)

            for pair in (2 * half, 2 * half + 1):
                # ---- QK projection + RoPE (layout b) ----
                wq = []
                for mb in range(4):
                    wt = wqk_pool.tile([128, 16, 128], DT, tag="wqk")
                    nc.sync.dma_start(out=wt, in_=w_qk[4 * pair + mb, :, :, :])
                    wq.append(wt)
                qe = qk_pool.tile([128, PH], DT, tag="qk")
                qo = qk_pool.tile([128, PH], DT, tag="qk")
                ke = qk_pool.tile([128, PH], DT, tag="qk")
                ko = qk_pool.tile([128, PH], DT, tag="qk")
                for (st, wd) in CHUNKS3:
                    csl = ds(st, wd)
                    for (mb_e, mb_o, dst_e, dst_o) in ((0, 1, qe, qo), (2, 3, ke, ko)):
                        pse = psA.tile([128, 512], F32, tag="ps")
                        pso = psA.tile([128, 512], F32, tag="ps")
                        for k in range(16):
                            nc.tensor.matmul(pse[:, :wd], lhsT=wq[mb_e][:, k, :],
                                             rhs=xt[k][:, csl],
                                             start=(k == 0), stop=(k == 15))
                        for k in range(16):
                            nc.tensor.matmul(pso[:, :wd], lhsT=wq[mb_o][:, k, :],
                                             rhs=xt[k][:, csl],
                                             start=(k == 0), stop=(k == 15))
                        ta = rt_pool.tile([128, 512], F32, tag="rt")
                        tb = rt_pool.tile([128, 512], F32, tag="rt")
                        nc.vector.tensor_mul(ta[:, :wd], pse[:, :wd], cos2[:, csl])
                        nc.vector.tensor_mul(tb[:, :wd], pso[:, :wd], sin2[:, csl])
                        nc.vector.tensor_sub(dst_e[:, csl], ta[:, :wd], tb[:, :wd])
                        tc_ = rt_pool.tile([128, 512], F32, tag="rt")
                        td = rt_pool.tile([128, 512], F32, tag="rt")
                        nc.vector.tensor_mul(tc_[:, :wd], pse[:, :wd], sin2[:, csl])
                        nc.vector.tensor_mul(td[:, :wd], pso[:, :wd], cos2[:, csl])
                        nc.vector.tensor_add(dst_o[:, csl], tc_[:, :wd], td[:, :wd])

                for sl in range(2):
                    h = 2 * pair + sl
                    hs = h % 4
                    pb = 64 * sl
                    # ---- z projection + gates ----
                    wz_sb = wz_pool.tile([128, 16, 128], DT, tag="wz")
                    nc.sync.dma_start(out=wz_sb, in_=w_z[h, :, :, :])
                    zps = psB.tile([128, 2, 512], F32, tag="z")
                    for c2 in range(2):
                        for k in range(16):
                            nc.tensor.matmul(zps[:, c2, :], lhsT=wz_sb[:, k, :],
                                             rhs=xt[k][:, ds(4 + c2 * 512, 512)],
                                             start=(k == 0), stop=(k == 15))
                    sigz = z_pool.tile([128, 2, 512], DT, tag="sigz")
                    silu = z_pool.tile([128, 2, 512], DT, tag="silu")
                    nc.scalar.activation(sigz, zps, AF.Sigmoid)
                    nc.vector.tensor_mul(silu, zps, sigz)

                    yn_h = yn_pool.tile([128, SC], DT, tag="yn")
                    for c in range(2):
                        agb = agb_pool.tile([128, 512], F32, tag="agb")
                        nc.sync.dma_start(
                            out=agb,
                            in_=ba_dram[8 + h:9 + h, ds(4 + c * 512, 512)]
                            .to_broadcast((128, 512)))
                        sps = psA.tile([128, 512], F32, tag="ps")
                        s2ps = psA.tile([4, 512], F32, tag="ps")
                        G = ag_pool.tile([128, 512], F32, tag="G")
                        G2 = ag_pool.tile([4, 512], F32, tag="G2")
                        for t_ in range(4):
                            t = 4 * c + t_
                            csl = ds(t_ * 128, 128)
                            jsl = ds(t * 128, 128)
                            j2 = ds(t * 128 + 128, 4)
                            isl = ds(4 + t * 128, 128)
                            nc.tensor.matmul(sps[:, csl], lhsT=ke[pb:pb + 64, jsl],
                                             rhs=qe[pb:pb + 64, isl],
                                             start=True, stop=False)
                            nc.tensor.matmul(sps[:, csl], lhsT=ko[pb:pb + 64, jsl],
                                             rhs=qo[pb:pb + 64, isl],
                                             start=False, stop=True)
                            nc.tensor.matmul(s2ps[:, csl], lhsT=ke[pb:pb + 64, j2],
                                             rhs=qe[pb:pb + 64, isl],
                                             start=True, stop=False)
                            nc.tensor.matmul(s2ps[:, csl], lhsT=ko[pb:pb + 64, j2],
                                             rhs=qo[pb:pb + 64, isl],
                                             start=False, stop=True)
                            nc.scalar.activation(G[:, csl], agb[:, csl], AF.Sigmoid,
                                                 scale=baat[t][:, h:h + 1])
                            nc.scalar.activation(G2[:, csl], agb[0:4, csl], AF.Sigmoid,
                                                 scale=baat[t + 1][0:4, h:h + 1])
                        A = ag_pool.tile([128, 512], DT, tag="A")
                        A2 = ag_pool.tile([4, 512], DT, tag="A2")
                        nc.vector.tensor_mul(A, sps, G)
                        nc.vector.tensor_mul(A, A, mask_c0 if c == 0 else mask_c1)
                        if c == 0:
                            nc.vector.tensor_add(A[:, 0:128], A[:, 0:128], mask_add)
                        nc.vector.tensor_mul(A2, s2ps, G2)
                        nc.vector.tensor_mul(A2, A2, mask2)
                        aps = psA.tile([128, 512], F32, tag="ps")
                        for t_ in range(4):
                            t = 4 * c + t_
                            csl = ds(t_ * 128, 128)
                            nc.tensor.matmul(aps[:, csl],
                                             lhsT=vtiles[t][:, ds(hs * 128, 128)],
                                             rhs=A[:, csl], start=True, stop=False)
                            nc.tensor.matmul(aps[:, csl],
                                             lhsT=vtiles[t + 1][0:4, ds(hs * 128, 128)],
                                             rhs=A2[:, csl], start=False, stop=True)
                        y = y_pool.tile([128, 512], F32, tag="y")
                        nc.vector.tensor_mul(y, aps, sigz[:, c, :])
                        ysq = y_pool.tile([128, 512], F32, tag="ysq")
                        nc.scalar.square(ysq, y)
                        nc.tensor.matmul(sum_ps[c], lhsT=ones, rhs=ysq,
                                         start=(h == 0), stop=(h == 7))
                        nc.vector.tensor_mul(yn_h[:, ds(c * 512, 512)], y,
                                             silu[:, c, :])
                    yn_tiles.append(yn_h)

        # ---- rstd ----
        srow = misc.tile([1, SC], F32, tag="srow")
        for c in range(2):
            nc.scalar.activation(srow[:, ds(c * 512, 512)], sum_ps[c], AF.Sqrt,
                                 bias=epsc, scale=1.0 / KD)
        rrow = misc.tile([1, SC], F32, tag="rrow")
        nc.vector.reciprocal(rrow, srow)
        rstd_b = misc.tile([128, SC], F32, tag="rstdb")
        nc.gpsimd.partition_broadcast(rstd_b, rrow)

        # ---- out projection, rstd fused at PSUM evacuation ----
        for hb in range(16):
            wo_sb = wo_pool.tile([128, 8, 128], DT, tag="wo")
            nc.sync.dma_start(out=wo_sb, in_=w_out[hb, :, :, :])
            for c in range(2):
                ops = psA.tile([128, 512], F32, tag="ps")
                for k in range(8):
                    nc.tensor.matmul(ops, lhsT=wo_sb[:, k, :],
                                     rhs=yn_tiles[k][:, ds(c * 512, 512)],
                                     start=(k == 0), stop=(k == 7))
                osb = o_pool.tile([128, 512], F32, tag="osb")
                nc.vector.tensor_mul(osb, ops, rstd_b[:, ds(c * 512, 512)])
                nc.sync.dma_start(out=outT[hb, :, ds(c * 512, 512)], in_=osb)

    nc.compile()
    return nc


@functools.lru_cache(maxsize=1)
def _get_nc():
    return build_program()


def _prep_core_inputs(x, w_qk_t, w_v_t, w_z_t, w_ba_t, w_out_t, core):
    b, q = core // 4, core % 4
    st = SC * q
    seg = np.zeros((PH, H), np.float32)
    lo = max(st - 4, 0)
    seg[4 - (st - lo):] = x[b, lo:st + SC]
    xT_c = np.ascontiguousarray(seg.T).reshape(16, 128, PH).astype(NPDT)

    # rope tables (absolute positions; halo positions < 0 get identity)
    inv = 1.0 / (ROPE_BASE ** (np.arange(0, D, 2)[: D // 2].astype(np.float32) / D))
    pos = (st - 4 + np.arange(PH)).astype(np.float32)
    f = np.outer(pos, inv)
    cosf = np.cos(f)
    sinf = np.sin(f)
    if q == 0:
        cosf[:4] = 1.0
        sinf[:4] = 0.0
    cos2 = np.ascontiguousarray(np.tile(cosf.T, (2, 1))).astype(np.float32)
    sin2 = np.ascontiguousarray(np.tile(sinf.T, (2, 1))).astype(np.float32)

    # band masks
    jj = np.arange(128)[:, None]          # j' within tile window
    ii = np.arange(512)[None, :] % 128    # i within tile
    band = ((ii <= jj) & (jj <= ii + 4)).astype(np.float32) * SCALE
    mask_c1 = band.copy()
    mask_c0 = band.copy()
    if q == 0:
        blk0 = slice(0, 128)
        m0 = mask_c0[:, blk0]
        m0[0:4, :] = 0.0      # j_global < 0
        m0[:, 0] = 0.0        # position-0 bypass: kill scored col
        mask_c0[:, blk0] = m0
    mask_add = np.zeros((128, 128), np.float32)
    if q == 0:
        mask_add[4, 0] = 1.0  # out[0] = v[0]
    jj2 = np.arange(4)[:, None]
    mask2 = ((np.arange(512)[None, :] % 128) >= 124 + jj2).astype(np.float32) * SCALE

    return {
        "xT": xT_c, "w_qk": w_qk_t, "w_v": w_v_t, "w_z": w_z_t,
        "w_ba": w_ba_t, "w_out": w_out_t,
        "cos2": cos2, "sin2": sin2,
        "mask_c0": mask_c0, "mask_c1": mask_c1,
        "mask_add": mask_add, "mask2": np.ascontiguousarray(mask2, np.float32),
    }


def prep_inputs(x, W_qkv, W_z, W_b, W_a, norm_w, W_out):
    x = np.asarray(x, np.float32)
    W_qkv = np.asarray(W_qkv, np.float32)

    Wq = W_qkv[:, :KD].reshape(H, NH, D)
    Wk = W_qkv[:, KD:2 * KD].reshape(H, NH, D)
    Wv = W_qkv[:, 2 * KD:]
    ev = np.arange(0, D, 2)
    od = ev + 1
    blocks = []
    for p in range(4):
        h0, h1 = 2 * p, 2 * p + 1
        blocks += [
            np.concatenate([Wq[:, h0, ev], Wq[:, h1, ev]], 1),
            np.concatenate([Wq[:, h0, od], Wq[:, h1, od]], 1),
            np.concatenate([Wk[:, h0, ev], Wk[:, h1, ev]], 1),
            np.concatenate([Wk[:, h0, od], Wk[:, h1, od]], 1),
        ]
    w_qk = np.concatenate(blocks, 1)  # [2048, 2048]
    w_qk_t = np.ascontiguousarray(
        w_qk.reshape(16, 128, 16, 128).transpose(2, 1, 0, 3)).astype(NPDT)
    w_v_t = np.ascontiguousarray(
        np.asarray(Wv).reshape(16, 128, 2, 512).transpose(2, 1, 0, 3)).astype(NPDT)
    w_z_t = np.ascontiguousarray(
        np.asarray(W_z, np.float32).reshape(16, 128, 8, 128).transpose(2, 1, 0, 3)
    ).astype(NPDT)
    w_ba = np.concatenate([np.asarray(W_b, np.float32),
                           np.asarray(W_a, np.float32)], 1)  # [2048, 16]
    w_ba_t = np.ascontiguousarray(
        w_ba.reshape(16, 128, 16).transpose(1, 0, 2)).astype(NPDT)
    w_out_f = np.asarray(norm_w, np.float32)[:, None] * np.asarray(W_out, np.float32)
    w_out_t = np.ascontiguousarray(
        w_out_f.reshape(8, 128, 16, 128).transpose(2, 1, 0, 3)).astype(NPDT)

    return [
        _prep_core_inputs(x, w_qk_t, w_v_t, w_z_t, w_ba_t, w_out_t, c)
        for c in range(NCORES)
    ]


def assemble_output(results):
    out = np.zeros((B, S, H), np.float32)
    for c in range(NCORES):
        b, q = c // 4, c % 4
        st = SC * q
        out[b, st:st + SC] = results[c]["outT"].reshape(H, SC).T
    return out


def kernel(x, W_qkv, W_z, W_b, W_a, norm_w, W_out):
    nc = _get_nc()
    in_maps = prep_inputs(x, W_qkv, W_z, W_b, W_a, norm_w, W_out)
    res = run_bass_kernel_spmd(nc, in_maps, core_ids=list(range(NCORES)))
    return assemble_output(res.results)

